# revision 1
# baseline (speedup 1.0000x reference)
"""Trainium2 Bass kernel for nn_CubicSpline (natural cubic spline radial eval).

Formulation: out[t, ch] = sum_s Theta[s, ch] * V_s(u_t), u = r/h, where the
V rows are truncated |.|-cubes  relu(w - |u - c|)^3  at two radii (w=2, w=1),
whose span contains the cubic B-spline bumps and hence every natural cubic
spline on the integer knot grid exactly (fp32 residual ~5e-7, |theta| <= ~6).

Device pipeline per 512-trial block (channel-major PSUM output), all fp32
(f32r was measured at ~1e-3 rel err vs 2.6e-5 for fp32 - rejected):
  PE   mm1: u_bcast[128,512] = (ones/h).T @ r_row      (K=1 fp32 matmul)
  DVE  passA: VA = relu(2 - |u - cA|)^3                (one custom op, 7 stages)
  DVE  passV: VV = relu(1 - |u - cV|)^3                (same op, other params)
  PE   out_psum = ThA.T @ VA + ThV.T @ VV              (2x K=128 fp32 matmuls)
  ACT  evict: out_sbuf = Identity(out_psum + bias)     (per-channel bias row)
  DMA  out_sbuf -> HBM shard [128, Nc] (channel-major; host transposes)

Data-parallel over 8 NeuronCores: r sharded along N, theta tables replicated.
"""

import os
import numpy as np

N_TOTAL = 2_000_000
N_CORES = 8
N_KNOTS = 128
RMAX = 6.0
H = RMAX / (N_KNOTS - 1)
BLK = 512
NC_RAW = N_TOTAL // N_CORES                 # 250_000
BLOCKS = (NC_RAW + BLK - 1) // BLK          # 489
NC_PAD = BLOCKS * BLK                       # 250_368
CHUNK_BLKS = 16
USE_GPSIMD_BCAST = False

_PROGRAM_CACHE = {}


def _register_op():
    from concourse import dve_ops
    from concourse.dve_spec import Spec, Src0, C0, C1, Zero, relu, sq, maxx, lower
    from concourse.dve_uop import DveOpSpec

    for o in dve_ops.OPS:
        if o.name == "BUMP3_ANT":
            return o
    t = Src0 - C0
    y = maxx(t, Zero - t)
    m = relu(C1 - y)
    spec = Spec(
        body=sq(m) * m,
        reference=lambda in0, s0, s1, imm2=0.0: np.maximum(
            s1 - np.abs(in0 - s0), 0.0
        ).astype(np.float32) ** 3,
    )
    op = dve_ops.DveOp("BUMP3_ANT", spec, subdim=False, uops_sha={})
    _append_op(dve_ops, op, spec, DveOpSpec, lower)
    return op


def _append_op(dve_ops, op, spec, DveOpSpec, lower):
    dve_ops.OPS.append(op)
    dve_ops._SUB_OPCODE_FOR_NAME[op.name] = (
        dve_ops._CUSTOM_DVE_ROW_BASE + len(dve_ops.OPS) - 1
    )
    dve_ops.CUSTOM_DVE_SPECS[op.name] = op.spec
    for ver in ("v3", "v4"):
        try:
            uops = lower(spec, ver=ver)
            op.uops_sha[ver] = DveOpSpec(
                name=op.name,
                opcode=dve_ops.get_dve_sub_opcode(op.name),
                uops=uops,
                rd1_en=False,
            ).sha(ver)
        except Exception:
            pass


def _register_op_scaled():
    """BUMP3S: m = relu(s1 - |in0*imm2 - s0|); out = m^3  (scale folded in)."""
    from concourse import dve_ops
    from concourse.dve_spec import Spec, Src0, C0, C1, C2, Zero, relu, sq, maxx, lower
    from concourse.dve_uop import DveOpSpec

    for o in dve_ops.OPS:
        if o.name == "BUMP3S_ANT":
            return o
    t = Src0 * C2 - C0
    y = maxx(t, Zero - t)
    m = relu(C1 - y)
    spec = Spec(
        body=sq(m) * m,
        reference=lambda in0, s0, s1, imm2: np.maximum(
            s1 - np.abs(in0 * imm2 - s0), 0.0
        ).astype(np.float32) ** 3,
    )
    op = dve_ops.DveOp("BUMP3S_ANT", spec, subdim=False, uops_sha={})
    _append_op(dve_ops, op, spec, DveOpSpec, lower)
    return op


# basis row centers (in u = r/h units)
CT_A = np.arange(-1, 127, dtype=np.float64)   # radius-2 rows, ct = -1..126
CT_V = np.arange(0, 128, dtype=np.float64)    # radius-1 rows, ct = 0..127


def _solve_theta(coefficients):
    """Fit bias + 256 cube rows to the spline defined by `coefficients`."""
    coef = np.asarray(coefficients, np.float64)           # [127, 4, 128]
    segs = np.arange(127)
    ts = (np.arange(8) + 0.5) / 8
    u = (segs[:, None] + ts[None, :]).ravel()             # [1016]
    idx = np.clip(np.floor(u).astype(int), 0, 126)
    dx = (u - idx) * H
    a, b, c, d = (coef[idx, k] for k in range(4))
    P = a + dx[:, None] * (b + dx[:, None] * (c + dx[:, None] * d))  # [1016,128]

    B = np.empty((u.size, 257))
    B[:, 0] = 1.0
    for i, ct in enumerate(CT_A):
        m = np.maximum(2.0 - np.abs(u - ct), 0.0)
        B[:, 1 + i] = m * m * m
    for i, ct in enumerate(CT_V):
        m = np.maximum(1.0 - np.abs(u - ct), 0.0)
        B[:, 129 + i] = m * m * m
    theta, _, _, _ = np.linalg.lstsq(B, P, rcond=None)
    bias = theta[0].astype(np.float32).reshape(128, 1)
    thA = theta[1:129].astype(np.float32)                 # [128 rows, 128 ch]
    thV = theta[129:257].astype(np.float32)
    return thA, thV, bias


def _build_program(n_blocks):
    if n_blocks in _PROGRAM_CACHE:
        return _PROGRAM_CACHE[n_blocks]
    import concourse.bacc as bacc
    import concourse.mybir as mybir
    from concourse.tile import TileContext

    op = _register_op()
    ops = _register_op_scaled()
    f32 = mybir.dt.float32
    f32r = mybir.dt.float32r
    nc = bacc.Bacc(
        "TRN2", target_bir_lowering=False, debug=False, num_devices=N_CORES
    )
    n_pad = n_blocks * BLK
    r_ap = nc.dram_tensor("r", [1, n_pad], f32, kind="ExternalInput").ap()
    thA_ap = nc.dram_tensor("thA", [128, 128], f32, kind="ExternalInput").ap()
    thV_ap = nc.dram_tensor("thV", [128, 128], f32, kind="ExternalInput").ap()
    bias_ap = nc.dram_tensor("bias", [128, 1], f32, kind="ExternalInput").ap()
    ctA_ap = nc.dram_tensor("ctA", [128, 1], f32, kind="ExternalInput").ap()
    ctV_ap = nc.dram_tensor("ctV", [128, 1], f32, kind="ExternalInput").ap()
    ones_ap = nc.dram_tensor("onesh", [1, 128], f32, kind="ExternalInput").ap()
    out_ap = nc.dram_tensor("out", [128, n_pad], f32, kind="ExternalOutput").ap()

    with TileContext(nc) as tc:
        with tc.tile_pool(name="const", bufs=1) as cpool, tc.tile_pool(
            name="work", bufs=3
        ) as pool, tc.tile_pool(name="rch", bufs=2) as rpool, tc.tile_pool(
            name="pu", bufs=2, space="PSUM"
        ) as ppool, tc.tile_pool(name="po", bufs=2, space="PSUM") as opool:
            thA_t = cpool.tile([128, 128], f32)
            nc.sync.dma_start(thA_t[:], thA_ap)
            thV_t = cpool.tile([128, 128], f32)
            nc.sync.dma_start(thV_t[:], thV_ap)
            bias_t = cpool.tile([128, 1], f32)
            nc.sync.dma_start(bias_t[:], bias_ap)
            ctA_t = cpool.tile([128, 1], f32)
            nc.sync.dma_start(ctA_t[:], ctA_ap)
            ctV_t = cpool.tile([128, 1], f32)
            nc.sync.dma_start(ctV_t[:], ctV_ap)
            ones_t = cpool.tile([1, 128], f32)
            nc.sync.dma_start(ones_t[:], ones_ap)

            for c0 in range(0, n_blocks, CHUNK_BLKS):
                bc = min(CHUNK_BLKS, n_blocks - c0)
                rch = rpool.tile([1, CHUNK_BLKS * BLK], f32, tag="rch")
                nc.sync.dma_start(
                    rch[:, : bc * BLK], r_ap[:, c0 * BLK : (c0 + bc) * BLK]
                )
                for b in range(bc):
                    rsl = rch[:, b * BLK : (b + 1) * BLK]
                    if USE_GPSIMD_BCAST:
                        pu = pool.tile([128, BLK], f32, tag="pu")
                        nc.gpsimd.partition_broadcast(pu[:], rsl, channels=128)
                        inv_h = float(np.float32(1.0) / np.float32(H))
                        va = pool.tile([128, BLK], f32, tag="va")
                        nc.vector._custom_dve(
                            ops, out=va[:], in0=pu[:], s0=ctA_t[:], s1=2.0, imm2=inv_h
                        )
                        vv = pool.tile([128, BLK], f32, tag="vv")
                        nc.vector._custom_dve(
                            ops, out=vv[:], in0=pu[:], s0=ctV_t[:], s1=1.0, imm2=inv_h
                        )
                    else:
                        pu = ppool.tile([128, BLK], f32, tag="pu")
                        nc.tensor.matmul(
                            pu[:],
                            ones_t[:],
                            rsl,
                            start=True,
                            stop=True,
                        )
                        va = pool.tile([128, BLK], f32, tag="va")
                        nc.vector._custom_dve(
                            op, out=va[:], in0=pu[:], s0=ctA_t[:], s1=2.0
                        )
                        vv = pool.tile([128, BLK], f32, tag="vv")
                        nc.vector._custom_dve(
                            op, out=vv[:], in0=pu[:], s0=ctV_t[:], s1=1.0
                        )
                    po = opool.tile([128, BLK], f32, tag="po")
                    nc.tensor.matmul(
                        po[:],
                        thA_t[:],
                        va[:],
                        start=True,
                        stop=False,
                    )
                    nc.tensor.matmul(
                        po[:],
                        thV_t[:],
                        vv[:],
                        start=False,
                        stop=True,
                    )
                    ob = pool.tile([128, BLK], f32, tag="ob")
                    nc.scalar.activation(
                        ob[:],
                        po[:],
                        mybir.ActivationFunctionType.Identity,
                        bias=bias_t[:],
                        scale=1.0,
                    )
                    blk = c0 + b
                    nc.sync.dma_start(out_ap[:, blk * BLK : (blk + 1) * BLK], ob[:])
    nc.compile()
    _PROGRAM_CACHE[n_blocks] = nc
    return nc


def kernel(r_trial, r_knots, coefficients, h, rmax):
    r = np.ascontiguousarray(np.asarray(r_trial, np.float32))
    n = r.shape[0]
    thA, thV, bias = _solve_theta(coefficients)
    inv_h = np.float32(1.0 / H)

    n_blocks = BLOCKS
    n_pad = NC_PAD
    r_pad = np.zeros(N_CORES * n_pad, np.float32)
    r_pad[:n] = r
    shards = r_pad.reshape(N_CORES, 1, n_pad)

    ctA32 = (CT_A.astype(np.float32)).reshape(128, 1)
    ctV32 = (CT_V.astype(np.float32)).reshape(128, 1)
    ones = np.full((1, 128), inv_h, np.float32)

    nc = _build_program(n_blocks)
    in_maps = [
        {
            "r": shards[i],
            "thA": thA,
            "thV": thV,
            "bias": bias,
            "ctA": ctA32,
            "ctV": ctV32,
            "onesh": ones,
        }
        for i in range(N_CORES)
    ]
    from concourse.bass_utils import run_bass_kernel_spmd

    res = run_bass_kernel_spmd(nc, in_maps, core_ids=list(range(N_CORES)))
    full = np.empty((N_CORES * n_pad, 128), np.float32)
    for i in range(N_CORES):
        full[i * n_pad : (i + 1) * n_pad] = res.results[i]["out"].T
    return full[:n]



# revision 3
# speedup vs baseline: 2.3308x; 2.3308x over previous
"""Trainium2 Bass kernel for nn_CubicSpline — histogram-binning formulation.

Host bins (stable-sorts) each core's trials by spline segment, then each
512-trial block touches at most 2 consecutive segments.  For block b the
device computes the exact reference Horner polynomial as ONE K=8 matmul:

  psum[128ch, 512] = W_b[8, 128]^T @ X_b[8, 512]
    W_b rows 0-3 = coefficients[segA(b), 0:4, :]   (a, b, c, d)
    W_b rows 4-7 = coefficients[segB(b), 0:4, :]
    X_b rows 0-3 = [1, dx, dx^2, dx^3] masked to segA's columns (else 0)
    X_b rows 4-7 = same masked to segB's columns

PE matmul cost is K-independent (ap_size * pe_cycle * cyc/row) and f32r at
ap_size>=512 runs 1 cycle/row, so the whole polynomial eval is one cheap
matmul per block.  Evict alternates ACT/DVE (psum f32 -> sbuf fp16), output
DMA is batched per 16-block chunk, and the host scatters rows back to the
original trial order (and upcasts fp16 -> f32).

Trials with r >= rmax (and pad columns) get all-zero X columns -> exact 0.
Any trial in a block with >2 segments (statistically impossible here, but
handled) is zero-masked on device and fixed up exactly on the host.
"""

import numpy as np

N_TOTAL = 2_000_000
N_CORES = 8
N_KNOTS = 128
N_SEG = N_KNOTS - 1
RMAX = 6.0
H = RMAX / N_SEG
BLK = 512
NC_PAD_RAW = (N_TOTAL // N_CORES)                  # 250_000
BLOCKS = (NC_PAD_RAW + BLK - 1) // BLK             # 489
NC_PAD = BLOCKS * BLK                              # 250_368
CHUNK_BLKS = 16
KDIM = 8

_PROGRAM_CACHE = {}


def _build_program(n_blocks):
    if n_blocks in _PROGRAM_CACHE:
        return _PROGRAM_CACHE[n_blocks]
    import concourse.bacc as bacc
    import concourse.mybir as mybir
    from concourse.tile import TileContext

    f32 = mybir.dt.float32
    f32r = mybir.dt.float32r
    f16 = mybir.dt.float16
    nc = bacc.Bacc(
        "TRN2", target_bir_lowering=False, debug=False, num_devices=N_CORES
    )
    n_pad = n_blocks * BLK
    x_ap = nc.dram_tensor("x", [KDIM, n_pad], f32r, kind="ExternalInput").ap()
    w_ap = nc.dram_tensor(
        "w", [KDIM, n_blocks * 128], f32r, kind="ExternalInput"
    ).ap()
    out_ap = nc.dram_tensor("out", [128, n_pad], f16, kind="ExternalOutput").ap()

    with TileContext(nc) as tc:
        with tc.tile_pool(name="xw", bufs=2) as xwpool, tc.tile_pool(
            name="ob", bufs=2
        ) as obpool, tc.tile_pool(name="ps", bufs=4, space="PSUM") as ppool:
            for c0 in range(0, n_blocks, CHUNK_BLKS):
                bc = min(CHUNK_BLKS, n_blocks - c0)
                xch = xwpool.tile([KDIM, CHUNK_BLKS * BLK], f32r, tag="x")
                nc.sync.dma_start(
                    xch[:, : bc * BLK], x_ap[:, c0 * BLK : (c0 + bc) * BLK]
                )
                wch = xwpool.tile([KDIM, CHUNK_BLKS * 128], f32r, tag="w")
                nc.sync.dma_start(
                    wch[:, : bc * 128], w_ap[:, c0 * 128 : (c0 + bc) * 128]
                )
                och = obpool.tile([128, CHUNK_BLKS * BLK], f16, tag="o")
                for b in range(bc):
                    po = ppool.tile([128, BLK], f32, tag="po")
                    nc.tensor.matmul(
                        po[:],
                        wch[:, b * 128 : (b + 1) * 128],
                        xch[:, b * BLK : (b + 1) * BLK],
                        start=True,
                        stop=True,
                    )
                    osl = och[:, b * BLK : (b + 1) * BLK]
                    if b % 2 == 0:
                        nc.scalar.activation(
                            osl, po[:], mybir.ActivationFunctionType.Copy
                        )
                    else:
                        nc.vector.tensor_scalar_add(osl, po[:], 0.0)
                nc.sync.dma_start(
                    out_ap[:, c0 * BLK : (c0 + bc) * BLK], och[:, : bc * BLK]
                )
    nc.compile()
    _PROGRAM_CACHE[n_blocks] = nc
    return nc


def kernel(r_trial, r_knots, coefficients, h, rmax):
    r = np.ascontiguousarray(np.asarray(r_trial, np.float32))
    rk = np.asarray(r_knots, np.float32)
    coef = np.asarray(coefficients, np.float32)          # [127, 4, 128]
    h32 = np.float32(h)
    rmax32 = np.float32(rmax)
    n = r.shape[0]

    total_pad = N_CORES * NC_PAD
    rp = np.zeros(total_pad, np.float32)
    rp[:n] = r
    valid = np.zeros(total_pad, bool)
    valid[:n] = r < rmax32

    # segment + local offset, reference float32 semantics
    t = (rp - rk[0]) / h32
    idx = np.clip(np.floor(t).astype(np.int32), 0, N_SEG - 1)
    dx = rp - rk[idx]

    nc = _build_program(BLOCKS)

    bcol = np.arange(NC_PAD, dtype=np.int64) // BLK      # block id per column
    in_maps = []
    orders = []
    uncovered_all = []
    for i in range(N_CORES):
        sl = slice(i * NC_PAD, (i + 1) * NC_PAD)
        idx_i, dx_i, val_i = idx[sl], dx[sl], valid[sl]
        key = np.where(val_i, idx_i, np.int32(1000))     # invalid/pad sort last
        order = np.argsort(key, kind="stable")
        sidx = idx_i[order]
        sdx = dx_i[order]
        sval = val_i[order]

        segA = sidx[0::BLK]                              # [BLOCKS]
        segB = sidx[BLK - 1 :: BLK]
        mA = sval & (sidx == segA[bcol])
        mB = sval & ~mA & (sidx == segB[bcol])
        uncovered = sval & ~mA & ~mB                     # >2 segs in a block
        uncovered_all.append(np.flatnonzero(uncovered))

        x8 = np.zeros((KDIM, NC_PAD), np.float32)
        dA = np.where(mA, sdx, np.float32(0))
        dB = np.where(mB, sdx, np.float32(0))
        x8[0] = mA
        x8[1] = dA
        x8[2] = dA * dA
        x8[3] = x8[2] * dA
        x8[4] = mB
        x8[5] = dB
        x8[6] = dB * dB
        x8[7] = x8[6] * dB

        w8 = np.empty((KDIM, BLOCKS, 128), np.float32)
        w8[0:4] = coef[segA].transpose(1, 0, 2)
        w8[4:8] = coef[segB].transpose(1, 0, 2)

        in_maps.append({"x": x8, "w": w8.reshape(KDIM, BLOCKS * 128)})
        orders.append(order)

    from concourse.bass_utils import run_bass_kernel_spmd

    res = run_bass_kernel_spmd(nc, in_maps, core_ids=list(range(N_CORES)))

    full = np.empty((total_pad, 128), np.float32)
    for i in range(N_CORES):
        shard = np.empty((NC_PAD, 128), np.float32)
        shard[orders[i]] = res.results[i]["out"].T.astype(np.float32)
        full[i * NC_PAD : (i + 1) * NC_PAD] = shard

    # exact host fixup for trials the device had to zero-mask (rare/never)
    for i in range(N_CORES):
        unc = uncovered_all[i]
        if unc.size:
            g = i * NC_PAD + orders[i][unc]  # original positions
            ri = rp[g]
            ii = idx[g]
            di = dx[g][:, None]
            cf = coef[ii]
            o = cf[:, 0] + di * (cf[:, 1] + di * (cf[:, 2] + di * cf[:, 3]))
            o[ri >= rmax32] = 0.0
            full[g] = o

    return full[:n]


# revision 4
# speedup vs baseline: 4.6418x; 1.9915x over previous
"""Trainium2 Bass kernel for nn_CubicSpline — histogram-binning formulation.

Host bins (stable-sorts) each core's trials by spline segment, then each
512-trial block touches at most 2 consecutive segments.  For block b the
device computes the exact reference Horner polynomial as ONE K=8 matmul:

  psum[128ch, 512] = W_b[8, 128]^T @ X_b[8, 512]
    W_b rows 0-3 = [a, b*h, c*h^2, d*h^3] of segA(b)   (h-scaled coeffs)
    W_b rows 4-7 = same for segB(b)
    X_b rows 0-3 = [1, u, u^2, u^3], u = dx/h in [0,1], masked to segA cols
    X_b rows 4-7 = same masked to segB's columns

PE matmul cost is K-independent (ap_size * pe_cycle * cyc/row) and fp16 at
ap_size 512 runs 1 cycle/row, so the whole polynomial eval is one cheap
matmul per block.  The normalized-u form keeps every X/W value O(1) so fp16
inputs lose <5e-4 relative accuracy.  Evict alternates ACT/DVE (psum f32 ->
sbuf fp16); DMA is batched per 16-block chunk with next-chunk input loads
issued BEFORE this chunk's compute so the big output store never starves
the input path (the DMA pool is serial).  Host scatters rows back to the
original trial order and upcasts fp16 -> f32.

Trials with r >= rmax (and pad columns) get all-zero X columns -> exact 0.
Any trial in a block with >2 segments (statistically impossible here, but
handled) is zero-masked on device and fixed up exactly on the host.
"""

import numpy as np

N_TOTAL = 2_000_000
N_CORES = 8
N_KNOTS = 128
N_SEG = N_KNOTS - 1
RMAX = 6.0
H = RMAX / N_SEG
BLK = 512
NC_PAD_RAW = (N_TOTAL // N_CORES)                  # 250_000
BLOCKS = (NC_PAD_RAW + BLK - 1) // BLK             # 489
NC_PAD = BLOCKS * BLK                              # 250_368
CHUNK_BLKS = 16
KDIM = 8

_PROGRAM_CACHE = {}


def _build_program(n_blocks):
    if n_blocks in _PROGRAM_CACHE:
        return _PROGRAM_CACHE[n_blocks]
    import concourse.bacc as bacc
    import concourse.mybir as mybir
    from concourse.tile import TileContext

    f32 = mybir.dt.float32
    f16 = mybir.dt.float16
    nc = bacc.Bacc(
        "TRN2", target_bir_lowering=False, debug=False, num_devices=N_CORES
    )
    n_pad = n_blocks * BLK
    x_ap = nc.dram_tensor("x", [KDIM, n_pad], f16, kind="ExternalInput").ap()
    w_ap = nc.dram_tensor(
        "w", [KDIM, n_blocks * 128], f16, kind="ExternalInput"
    ).ap()
    out_ap = nc.dram_tensor("out", [128, n_pad], f16, kind="ExternalOutput").ap()

    chunks = [
        (c0, min(CHUNK_BLKS, n_blocks - c0))
        for c0 in range(0, n_blocks, CHUNK_BLKS)
    ]

    with TileContext(nc) as tc:
        with tc.tile_pool(name="xw", bufs=3) as xwpool, tc.tile_pool(
            name="ob", bufs=2
        ) as obpool, tc.tile_pool(name="ps", bufs=4, space="PSUM") as ppool:
            xtiles = {}
            wtiles = {}

            def load_chunk(k):
                c0, bc = chunks[k]
                xch = xwpool.tile([KDIM, CHUNK_BLKS * BLK], f16, tag="x")
                nc.sync.dma_start(
                    xch[:, : bc * BLK], x_ap[:, c0 * BLK : (c0 + bc) * BLK]
                )
                wch = xwpool.tile([KDIM, CHUNK_BLKS * 128], f16, tag="w")
                nc.sync.dma_start(
                    wch[:, : bc * 128], w_ap[:, c0 * 128 : (c0 + bc) * 128]
                )
                xtiles[k], wtiles[k] = xch, wch

            load_chunk(0)
            for k, (c0, bc) in enumerate(chunks):
                if k + 1 < len(chunks):
                    load_chunk(k + 1)   # prefetch before compute: keeps the
                    # serial DMA pool feeding inputs ahead of the big store
                xch, wch = xtiles.pop(k), wtiles.pop(k)
                och = obpool.tile([128, CHUNK_BLKS * BLK], f16, tag="o")
                for b in range(bc):
                    po = ppool.tile([128, BLK], f32, tag="po")
                    nc.tensor.matmul(
                        po[:],
                        wch[:, b * 128 : (b + 1) * 128],
                        xch[:, b * BLK : (b + 1) * BLK],
                        start=True,
                        stop=True,
                    )
                    osl = och[:, b * BLK : (b + 1) * BLK]
                    if b % 2 == 0:
                        nc.scalar.activation(
                            osl, po[:], mybir.ActivationFunctionType.Copy
                        )
                    else:
                        nc.vector.tensor_scalar_add(osl, po[:], 0.0)
                nc.sync.dma_start(
                    out_ap[:, c0 * BLK : (c0 + bc) * BLK], och[:, : bc * BLK]
                )
    nc.compile()
    _PROGRAM_CACHE[n_blocks] = nc
    return nc


def kernel(r_trial, r_knots, coefficients, h, rmax):
    r = np.ascontiguousarray(np.asarray(r_trial, np.float32))
    rk = np.asarray(r_knots, np.float32)
    coef = np.asarray(coefficients, np.float32)          # [127, 4, 128]
    h32 = np.float32(h)
    rmax32 = np.float32(rmax)
    n = r.shape[0]

    total_pad = N_CORES * NC_PAD
    rp = np.zeros(total_pad, np.float32)
    rp[:n] = r
    valid = np.zeros(total_pad, bool)
    valid[:n] = r < rmax32

    # segment + normalized local offset, reference float32 semantics
    t = (rp - rk[0]) / h32
    idx = np.clip(np.floor(t).astype(np.int32), 0, N_SEG - 1)
    dx = rp - rk[idx]
    u = dx / h32                                         # in [0, ~1]

    # h-scaled coefficients so every matmul operand is O(1):
    # out = a + (b*h)*u + (c*h^2)*u^2 + (d*h^3)*u^3
    hk = np.array([1.0, float(h32), float(h32) ** 2, float(h32) ** 3])
    coef_s = (coef.astype(np.float64) * hk[None, :, None]).astype(np.float32)

    nc = _build_program(BLOCKS)

    bcol = np.arange(NC_PAD, dtype=np.int64) // BLK      # block id per column
    in_maps = []
    orders = []
    uncovered_all = []
    for i in range(N_CORES):
        sl = slice(i * NC_PAD, (i + 1) * NC_PAD)
        idx_i, u_i, val_i = idx[sl], u[sl], valid[sl]
        key = np.where(val_i, idx_i, np.int32(1000))     # invalid/pad sort last
        order = np.argsort(key, kind="stable")
        sidx = idx_i[order]
        su = u_i[order]
        sval = val_i[order]

        segA = sidx[0::BLK]                              # [BLOCKS]
        segB = sidx[BLK - 1 :: BLK]
        mA = sval & (sidx == segA[bcol])
        mB = sval & ~mA & (sidx == segB[bcol])
        uncovered = sval & ~mA & ~mB                     # >2 segs in a block
        uncovered_all.append(np.flatnonzero(uncovered))

        x8 = np.zeros((KDIM, NC_PAD), np.float32)
        uA = np.where(mA, su, np.float32(0))
        uB = np.where(mB, su, np.float32(0))
        x8[0] = mA
        x8[1] = uA
        x8[2] = uA * uA
        x8[3] = x8[2] * uA
        x8[4] = mB
        x8[5] = uB
        x8[6] = uB * uB
        x8[7] = x8[6] * uB

        w8 = np.empty((KDIM, BLOCKS, 128), np.float32)
        w8[0:4] = coef_s[segA].transpose(1, 0, 2)
        w8[4:8] = coef_s[segB].transpose(1, 0, 2)

        in_maps.append(
            {
                "x": x8.astype(np.float16),
                "w": w8.reshape(KDIM, BLOCKS * 128).astype(np.float16),
            }
        )
        orders.append(order)

    from concourse.bass_utils import run_bass_kernel_spmd

    res = run_bass_kernel_spmd(nc, in_maps, core_ids=list(range(N_CORES)))

    full = np.empty((total_pad, 128), np.float32)
    for i in range(N_CORES):
        shard = np.empty((NC_PAD, 128), np.float32)
        shard[orders[i]] = res.results[i]["out"].T.astype(np.float32)
        full[i * NC_PAD : (i + 1) * NC_PAD] = shard

    # exact host fixup for trials the device had to zero-mask (rare/never)
    for i in range(N_CORES):
        unc = uncovered_all[i]
        if unc.size:
            g = i * NC_PAD + orders[i][unc]  # original positions
            ri = rp[g]
            ii = idx[g]
            di = dx[g][:, None]
            cf = coef[ii]
            o = cf[:, 0] + di * (cf[:, 1] + di * (cf[:, 2] + di * cf[:, 3]))
            o[ri >= rmax32] = 0.0
            full[g] = o

    return full[:n]


# revision 6
# speedup vs baseline: 6.4733x; 1.3946x over previous
"""Trainium2 Bass kernel for nn_CubicSpline — histogram-binning formulation.

Host bins (stable-sorts) each core's trials by spline segment, then each
512-trial block touches at most 2 consecutive segments.  For block b the
device computes the exact reference Horner polynomial as ONE K=8 matmul:

  psum[128ch, 512] = W_b[8, 128]^T @ X_b[8, 512]
    W_b rows 0-3 = [a, b*h, c*h^2, d*h^3] of segA(b)   (h-scaled coeffs)
    W_b rows 4-7 = same for segB(b)
    X_b rows 0-3 = [1, u, u^2, u^3], u = dx/h in [0,1], masked to segA cols
    X_b rows 4-7 = same masked to segB's columns

PE matmul cost is K-independent (ap_size * pe_cycle * cyc/row) and fp16 at
ap_size 512 runs 1 cycle/row, so the whole polynomial eval is one cheap
matmul per block.  The normalized-u form keeps every X/W value O(1) so fp16
inputs lose <5e-4 relative accuracy.  Evict alternates ACT/DVE (psum f32 ->
sbuf fp16); DMA is batched per 16-block chunk with next-chunk input loads
issued BEFORE this chunk's compute so the big output store never starves
the input path (the DMA pool is serial).  Host scatters rows back to the
original trial order and upcasts fp16 -> f32.

Trials with r >= rmax (and pad columns) get all-zero X columns -> exact 0.
Any trial in a block with >2 segments (statistically impossible here, but
handled) is zero-masked on device and fixed up exactly on the host.
"""

import numpy as np

N_TOTAL = 2_000_000
N_CORES = 8
N_KNOTS = 128
N_SEG = N_KNOTS - 1
RMAX = 6.0
H = RMAX / N_SEG
BLK = 512
NC_PAD_RAW = (N_TOTAL // N_CORES)                  # 250_000
BLOCKS = (NC_PAD_RAW + BLK - 1) // BLK             # 489
NC_PAD = BLOCKS * BLK                              # 250_368
CHUNK_BLKS = 16
KDIM = 8

_PROGRAM_CACHE = {}


def _build_program(n_blocks):
    if n_blocks in _PROGRAM_CACHE:
        return _PROGRAM_CACHE[n_blocks]
    import concourse.bacc as bacc
    import concourse.mybir as mybir
    from concourse.tile import TileContext

    f32 = mybir.dt.float32
    f16 = mybir.dt.float16
    nc = bacc.Bacc(
        "TRN2", target_bir_lowering=False, debug=False, num_devices=N_CORES
    )
    n_pad = n_blocks * BLK
    x_ap = nc.dram_tensor("x", [KDIM, n_pad], f16, kind="ExternalInput").ap()
    w_ap = nc.dram_tensor(
        "w", [KDIM, n_blocks * 128], f16, kind="ExternalInput"
    ).ap()
    out_ap = nc.dram_tensor("out", [128, n_pad], f16, kind="ExternalOutput").ap()

    chunks = [
        (c0, min(CHUNK_BLKS, n_blocks - c0))
        for c0 in range(0, n_blocks, CHUNK_BLKS)
    ]

    with TileContext(nc) as tc:
        with tc.tile_pool(name="xw", bufs=3) as xwpool, tc.tile_pool(
            name="ob", bufs=2
        ) as obpool, tc.tile_pool(name="ps", bufs=4, space="PSUM") as ppool:
            xtiles = {}
            wtiles = {}

            def load_chunk(k):
                c0, bc = chunks[k]
                xch = xwpool.tile([KDIM, CHUNK_BLKS * BLK], f16, tag="x")
                nc.sync.dma_start(
                    xch[:, : bc * BLK], x_ap[:, c0 * BLK : (c0 + bc) * BLK]
                )
                wch = xwpool.tile([KDIM, CHUNK_BLKS * 128], f16, tag="w")
                nc.sync.dma_start(
                    wch[:, : bc * 128], w_ap[:, c0 * 128 : (c0 + bc) * 128]
                )
                xtiles[k], wtiles[k] = xch, wch

            load_chunk(0)
            for k, (c0, bc) in enumerate(chunks):
                if k + 1 < len(chunks):
                    load_chunk(k + 1)   # prefetch before compute: keeps the
                    # serial DMA pool feeding inputs ahead of the big store
                xch, wch = xtiles.pop(k), wtiles.pop(k)
                och = obpool.tile([128, CHUNK_BLKS * BLK], f16, tag="o")
                half = (bc + 1) // 2
                for b in range(bc):
                    po = ppool.tile([128, BLK], f32, tag="po")
                    nc.tensor.matmul(
                        po[:],
                        wch[:, b * 128 : (b + 1) * 128],
                        xch[:, b * BLK : (b + 1) * BLK],
                        start=True,
                        stop=True,
                    )
                    osl = och[:, b * BLK : (b + 1) * BLK]
                    if b % 2 == 0:
                        nc.scalar.activation(
                            osl, po[:], mybir.ActivationFunctionType.Copy
                        )
                    else:
                        nc.vector.tensor_scalar_add(osl, po[:], 0.0)
                    if b == half - 1:
                        nc.sync.dma_start(
                            out_ap[:, c0 * BLK : (c0 + half) * BLK],
                            och[:, : half * BLK],
                        )
                nc.sync.dma_start(
                    out_ap[:, (c0 + half) * BLK : (c0 + bc) * BLK],
                    och[:, half * BLK : bc * BLK],
                )
    nc.compile()
    _PROGRAM_CACHE[n_blocks] = nc
    return nc


def kernel(r_trial, r_knots, coefficients, h, rmax):
    r = np.ascontiguousarray(np.asarray(r_trial, np.float32))
    rk = np.asarray(r_knots, np.float32)
    coef = np.asarray(coefficients, np.float32)          # [127, 4, 128]
    h32 = np.float32(h)
    rmax32 = np.float32(rmax)
    n = r.shape[0]

    total_pad = N_CORES * NC_PAD
    rp = np.zeros(total_pad, np.float32)
    rp[:n] = r
    valid = np.zeros(total_pad, bool)
    valid[:n] = r < rmax32

    # segment + normalized local offset, reference float32 semantics
    t = (rp - rk[0]) / h32
    idx = np.clip(np.floor(t).astype(np.int32), 0, N_SEG - 1)
    dx = rp - rk[idx]
    u = dx / h32                                         # in [0, ~1]

    # h-scaled coefficients so every matmul operand is O(1):
    # out = a + (b*h)*u + (c*h^2)*u^2 + (d*h^3)*u^3
    hk = np.array([1.0, float(h32), float(h32) ** 2, float(h32) ** 3])
    coef_s = (coef.astype(np.float64) * hk[None, :, None]).astype(np.float32)

    nc = _build_program(BLOCKS)

    bcol = np.arange(NC_PAD, dtype=np.int64) // BLK      # block id per column
    in_maps = []
    orders = []
    uncovered_all = []
    for i in range(N_CORES):
        sl = slice(i * NC_PAD, (i + 1) * NC_PAD)
        idx_i, u_i, val_i = idx[sl], u[sl], valid[sl]
        key = np.where(val_i, idx_i, np.int32(1000))     # invalid/pad sort last
        order = np.argsort(key, kind="stable")
        sidx = idx_i[order]
        su = u_i[order]
        sval = val_i[order]

        segA = sidx[0::BLK]                              # [BLOCKS]
        segB = sidx[BLK - 1 :: BLK]
        mA = sval & (sidx == segA[bcol])
        mB = sval & ~mA & (sidx == segB[bcol])
        uncovered = sval & ~mA & ~mB                     # >2 segs in a block
        uncovered_all.append(np.flatnonzero(uncovered))

        x8 = np.zeros((KDIM, NC_PAD), np.float32)
        uA = np.where(mA, su, np.float32(0))
        uB = np.where(mB, su, np.float32(0))
        x8[0] = mA
        x8[1] = uA
        x8[2] = uA * uA
        x8[3] = x8[2] * uA
        x8[4] = mB
        x8[5] = uB
        x8[6] = uB * uB
        x8[7] = x8[6] * uB

        w8 = np.empty((KDIM, BLOCKS, 128), np.float32)
        w8[0:4] = coef_s[segA].transpose(1, 0, 2)
        w8[4:8] = coef_s[segB].transpose(1, 0, 2)

        in_maps.append(
            {
                "x": x8.astype(np.float16),
                "w": w8.reshape(KDIM, BLOCKS * 128).astype(np.float16),
            }
        )
        orders.append(order)

    from concourse.bass_utils import run_bass_kernel_spmd

    res = run_bass_kernel_spmd(nc, in_maps, core_ids=list(range(N_CORES)))

    full = np.empty((total_pad, 128), np.float32)
    for i in range(N_CORES):
        shard = np.empty((NC_PAD, 128), np.float32)
        shard[orders[i]] = res.results[i]["out"].T.astype(np.float32)
        full[i * NC_PAD : (i + 1) * NC_PAD] = shard

    # exact host fixup for trials the device had to zero-mask (rare/never)
    for i in range(N_CORES):
        unc = uncovered_all[i]
        if unc.size:
            g = i * NC_PAD + orders[i][unc]  # original positions
            ri = rp[g]
            ii = idx[g]
            di = dx[g][:, None]
            cf = coef[ii]
            o = cf[:, 0] + di * (cf[:, 1] + di * (cf[:, 2] + di * cf[:, 3]))
            o[ri >= rmax32] = 0.0
            full[g] = o

    return full[:n]


# revision 7
# speedup vs baseline: 6.6279x; 1.0239x over previous
"""Trainium2 Bass kernel for nn_CubicSpline — histogram-binning formulation.

Host bins (stable-sorts) each core's trials by spline segment; each 512-trial
block then touches at most 2 *consecutive* segments A, B=A+1.  Because the
spline is C^2, the two segment cubics differ only by q*(z)^3 with a triple
root at the shared knot (q = (dA-dB)*h^3 per channel), so with z = distance
from segment B's left knot the whole block is ONE K=5 matmul:

  psum[128ch, 512] = W_b[5, 128]^T @ X_b[5, 512]
    X_b rows = [1, z, z^2, z^3, min(z,0)^3] * valid_mask      (z in [-1, 1])
    W_b rows = [aB, bB*h, cB*h^2, dB*h^3, (dA-dB)*h^3]        (all O(1))

PE matmul cost is K-independent (ap_size * pe_cycle * cyc/row) and fp16 at
ap_size 512 runs 1 cycle/row, so the polynomial eval is one cheap matmul per
block; the normalized form keeps X/W fp16-safe.  Evict alternates ACT/DVE
(psum f32 -> sbuf fp16); output DMA goes in half-chunk pieces and next-chunk
input loads are issued BEFORE each chunk's compute so the serial DMA pool
never starves the input path.  Small warmup chunks fill the pipeline fast.
Host scatters rows back to original trial order and upcasts fp16 -> f32.

Trials with r >= rmax (and pad columns) get all-zero X columns -> exact 0.
Blocks with >2 segments or non-adjacent segments (statistically impossible
here, but handled) get those trials zero-masked and exactly fixed up on host.
"""

import numpy as np

N_TOTAL = 2_000_000
N_CORES = 8
N_KNOTS = 128
N_SEG = N_KNOTS - 1
RMAX = 6.0
H = RMAX / N_SEG
BLK = 512
NC_PAD_RAW = (N_TOTAL // N_CORES)                  # 250_000
BLOCKS = (NC_PAD_RAW + BLK - 1) // BLK             # 489
NC_PAD = BLOCKS * BLK                              # 250_368
KDIM = 5


def _chunk_plan(n_blocks):
    """Small warmup chunks, then 16-block steady state."""
    sizes = [4, 4, 8]
    out, c0 = [], 0
    for s in sizes:
        if c0 + s > n_blocks:
            break
        out.append((c0, s))
        c0 += s
    while c0 < n_blocks:
        s = min(16, n_blocks - c0)
        out.append((c0, s))
        c0 += s
    return out


_PROGRAM_CACHE = {}


def _build_program(n_blocks):
    if n_blocks in _PROGRAM_CACHE:
        return _PROGRAM_CACHE[n_blocks]
    import concourse.bacc as bacc
    import concourse.mybir as mybir
    from concourse.tile import TileContext

    f32 = mybir.dt.float32
    f16 = mybir.dt.float16
    nc = bacc.Bacc(
        "TRN2", target_bir_lowering=False, debug=False, num_devices=N_CORES
    )
    n_pad = n_blocks * BLK
    x_ap = nc.dram_tensor("x", [KDIM, n_pad], f16, kind="ExternalInput").ap()
    w_ap = nc.dram_tensor(
        "w", [KDIM, n_blocks * 128], f16, kind="ExternalInput"
    ).ap()
    out_ap = nc.dram_tensor("out", [128, n_pad], f16, kind="ExternalOutput").ap()

    chunks = _chunk_plan(n_blocks)
    max_blks = max(bc for _, bc in chunks)

    with TileContext(nc) as tc:
        with tc.tile_pool(name="xw", bufs=3) as xwpool, tc.tile_pool(
            name="ob", bufs=2
        ) as obpool, tc.tile_pool(name="ps", bufs=4, space="PSUM") as ppool:
            xtiles = {}
            wtiles = {}

            def load_chunk(k):
                c0, bc = chunks[k]
                xch = xwpool.tile([KDIM, max_blks * BLK], f16, tag="x")
                nc.sync.dma_start(
                    xch[:, : bc * BLK], x_ap[:, c0 * BLK : (c0 + bc) * BLK]
                )
                wch = xwpool.tile([KDIM, max_blks * 128], f16, tag="w")
                nc.sync.dma_start(
                    wch[:, : bc * 128], w_ap[:, c0 * 128 : (c0 + bc) * 128]
                )
                xtiles[k], wtiles[k] = xch, wch

            load_chunk(0)
            for k, (c0, bc) in enumerate(chunks):
                if k + 1 < len(chunks):
                    load_chunk(k + 1)   # prefetch before compute: keeps the
                    # serial DMA pool feeding inputs ahead of the big store
                xch, wch = xtiles.pop(k), wtiles.pop(k)
                och = obpool.tile([128, max_blks * BLK], f16, tag="o")
                half = (bc + 1) // 2
                for b in range(bc):
                    po = ppool.tile([128, BLK], f32, tag="po")
                    nc.tensor.matmul(
                        po[:],
                        wch[:, b * 128 : (b + 1) * 128],
                        xch[:, b * BLK : (b + 1) * BLK],
                        start=True,
                        stop=True,
                    )
                    osl = och[:, b * BLK : (b + 1) * BLK]
                    if b % 2 == 0:
                        nc.scalar.activation(
                            osl, po[:], mybir.ActivationFunctionType.Copy
                        )
                    else:
                        nc.vector.tensor_scalar_add(osl, po[:], 0.0)
                    if b == half - 1:
                        nc.sync.dma_start(
                            out_ap[:, c0 * BLK : (c0 + half) * BLK],
                            och[:, : half * BLK],
                        )
                nc.sync.dma_start(
                    out_ap[:, (c0 + half) * BLK : (c0 + bc) * BLK],
                    och[:, half * BLK : bc * BLK],
                )
    nc.compile()
    _PROGRAM_CACHE[n_blocks] = nc
    return nc


def kernel(r_trial, r_knots, coefficients, h, rmax):
    r = np.ascontiguousarray(np.asarray(r_trial, np.float32))
    rk = np.asarray(r_knots, np.float32)
    coef = np.asarray(coefficients, np.float32)          # [127, 4, 128]
    h32 = np.float32(h)
    rmax32 = np.float32(rmax)
    n = r.shape[0]

    total_pad = N_CORES * NC_PAD
    rp = np.zeros(total_pad, np.float32)
    rp[:n] = r
    valid = np.zeros(total_pad, bool)
    valid[:n] = r < rmax32

    # segment + normalized local offset, reference float32 semantics
    t = (rp - rk[0]) / h32
    idx = np.clip(np.floor(t).astype(np.int32), 0, N_SEG - 1)
    dx = rp - rk[idx]
    u = dx / h32                                         # in [0, ~1]

    # h-scaled coefficients so every matmul operand is O(1):
    # out = a + (b*h)*u + (c*h^2)*u^2 + (d*h^3)*u^3
    hk = np.array([1.0, float(h32), float(h32) ** 2, float(h32) ** 3])
    coef_s = (coef.astype(np.float64) * hk[None, :, None]).astype(np.float32)

    nc = _build_program(BLOCKS)

    bcol = np.arange(NC_PAD, dtype=np.int64) // BLK      # block id per column
    in_maps = []
    orders = []
    uncovered_all = []
    for i in range(N_CORES):
        sl = slice(i * NC_PAD, (i + 1) * NC_PAD)
        idx_i, u_i, val_i = idx[sl], u[sl], valid[sl]
        key = np.where(val_i, idx_i, np.int32(1000))     # invalid/pad sort last
        order = np.argsort(key, kind="stable")
        sidx = idx_i[order]
        su = u_i[order]
        sval = val_i[order]

        segA = sidx[0::BLK]                              # [BLOCKS]
        segB = sidx[BLK - 1 :: BLK]
        mB = sval & (sidx == segB[bcol])
        mA = sval & ~mB & (sidx == segA[bcol]) & (segB[bcol] == segA[bcol] + 1)
        uncovered = sval & ~mA & ~mB          # >2 segs or non-adjacent
        uncovered_all.append(np.flatnonzero(uncovered))

        ok = mA | mB
        # z = u - (segB - seg) : 0-based from segment B's left knot
        z = np.where(ok, su + (sidx - segB[bcol]).astype(np.float32), 0.0)
        z = z.astype(np.float32)
        zm = np.minimum(z, np.float32(0))
        x5 = np.empty((KDIM, NC_PAD), np.float32)
        x5[0] = ok
        x5[1] = z
        x5[2] = z * z
        x5[3] = x5[2] * z
        x5[4] = zm * zm * zm

        cB = coef_s[segB]                                # [BLOCKS, 4, 128]
        w5 = np.empty((KDIM, BLOCKS, 128), np.float32)
        w5[0:4] = cB.transpose(1, 0, 2)
        w5[4] = coef_s[segA, 3] - cB[:, 3]               # (dA-dB)*h^3

        in_maps.append(
            {
                "x": x5.astype(np.float16),
                "w": w5.reshape(KDIM, BLOCKS * 128).astype(np.float16),
            }
        )
        orders.append(order)

    from concourse.bass_utils import run_bass_kernel_spmd

    res = run_bass_kernel_spmd(nc, in_maps, core_ids=list(range(N_CORES)))

    full = np.empty((total_pad, 128), np.float32)
    for i in range(N_CORES):
        shard = np.empty((NC_PAD, 128), np.float32)
        shard[orders[i]] = res.results[i]["out"].T.astype(np.float32)
        full[i * NC_PAD : (i + 1) * NC_PAD] = shard

    # exact host fixup for trials the device had to zero-mask (rare/never)
    for i in range(N_CORES):
        unc = uncovered_all[i]
        if unc.size:
            g = i * NC_PAD + orders[i][unc]  # original positions
            ri = rp[g]
            ii = idx[g]
            di = dx[g][:, None]
            cf = coef[ii]
            o = cf[:, 0] + di * (cf[:, 1] + di * (cf[:, 2] + di * cf[:, 3]))
            o[ri >= rmax32] = 0.0
            full[g] = o

    return full[:n]


# revision 8
# speedup vs baseline: 6.6508x; 1.0034x over previous
"""Trainium2 Bass kernel for nn_CubicSpline — histogram-binning formulation.

Host bins (stable-sorts) each core's trials by spline segment; each 512-trial
block then touches at most 2 *consecutive* segments A, B=A+1.  Because the
spline is C^2, the two segment cubics differ only by q*(z)^3 with a triple
root at the shared knot (q = (dA-dB)*h^3 per channel), so with z = distance
from segment B's left knot the whole block is ONE K=5 matmul:

  psum[128ch, 512] = W_b[5, 128]^T @ X_b[5, 512]
    X_b rows = [1, z, z^2, z^3, min(z,0)^3] * valid_mask      (z in [-1, 1])
    W_b rows = [aB, bB*h, cB*h^2, dB*h^3, (dA-dB)*h^3]        (all O(1))

PE matmul cost is K-independent (ap_size * pe_cycle * cyc/row) and fp16 at
ap_size 512 runs 1 cycle/row, so the polynomial eval is one cheap matmul per
block; the normalized form keeps X/W fp16-safe.  Evict alternates ACT/DVE
(psum f32 -> sbuf fp16); output DMA goes in half-chunk pieces and next-chunk
input loads are issued BEFORE each chunk's compute so the serial DMA pool
never starves the input path.  Small warmup chunks fill the pipeline fast.
Host scatters rows back to original trial order and upcasts fp16 -> f32.

Trials with r >= rmax (and pad columns) get all-zero X columns -> exact 0.
Blocks with >2 segments or non-adjacent segments (statistically impossible
here, but handled) get those trials zero-masked and exactly fixed up on host.
"""

import numpy as np

N_TOTAL = 2_000_000
N_CORES = 8
N_KNOTS = 128
N_SEG = N_KNOTS - 1
RMAX = 6.0
H = RMAX / N_SEG
BLK = 512
NC_PAD_RAW = (N_TOTAL // N_CORES)                  # 250_000
BLOCKS = (NC_PAD_RAW + BLK - 1) // BLK             # 489
NC_PAD = BLOCKS * BLK                              # 250_368
KDIM = 5


def _chunk_plan(n_blocks):
    """Small warmup chunks, then 16-block steady state."""
    sizes = [2, 2, 4, 8]
    out, c0 = [], 0
    for s in sizes:
        if c0 + s > n_blocks:
            break
        out.append((c0, s))
        c0 += s
    while c0 < n_blocks:
        s = min(16, n_blocks - c0)
        out.append((c0, s))
        c0 += s
    return out


_PROGRAM_CACHE = {}


def _build_program(n_blocks):
    if n_blocks in _PROGRAM_CACHE:
        return _PROGRAM_CACHE[n_blocks]
    import concourse.bacc as bacc
    import concourse.mybir as mybir
    from concourse.tile import TileContext

    f32 = mybir.dt.float32
    f16 = mybir.dt.float16
    nc = bacc.Bacc(
        "TRN2", target_bir_lowering=False, debug=False, num_devices=N_CORES
    )
    n_pad = n_blocks * BLK
    x_ap = nc.dram_tensor("x", [KDIM, n_pad], f16, kind="ExternalInput").ap()
    w_ap = nc.dram_tensor(
        "w", [KDIM, n_blocks * 128], f16, kind="ExternalInput"
    ).ap()
    out_ap = nc.dram_tensor("out", [128, n_pad], f16, kind="ExternalOutput").ap()

    chunks = _chunk_plan(n_blocks)
    max_blks = max(bc for _, bc in chunks)

    with TileContext(nc) as tc:
        with tc.tile_pool(name="xw", bufs=3) as xwpool, tc.tile_pool(
            name="ob", bufs=2
        ) as obpool, tc.tile_pool(name="ps", bufs=4, space="PSUM") as ppool:
            xtiles = {}
            wtiles = {}

            def load_chunk(k):
                c0, bc = chunks[k]
                xch = xwpool.tile([KDIM, max_blks * BLK], f16, tag="x")
                nc.sync.dma_start(
                    xch[:, : bc * BLK], x_ap[:, c0 * BLK : (c0 + bc) * BLK]
                )
                wch = xwpool.tile([KDIM, max_blks * 128], f16, tag="w")
                nc.gpsimd.dma_start(
                    wch[:, : bc * 128], w_ap[:, c0 * 128 : (c0 + bc) * 128]
                )
                xtiles[k], wtiles[k] = xch, wch

            load_chunk(0)
            for k, (c0, bc) in enumerate(chunks):
                if k + 1 < len(chunks):
                    load_chunk(k + 1)   # prefetch before compute: keeps the
                    # serial DMA pool feeding inputs ahead of the big store
                xch, wch = xtiles.pop(k), wtiles.pop(k)
                och = obpool.tile([128, max_blks * BLK], f16, tag="o")
                half = (bc + 1) // 2
                for b in range(bc):
                    po = ppool.tile([128, BLK], f32, tag="po")
                    nc.tensor.matmul(
                        po[:],
                        wch[:, b * 128 : (b + 1) * 128],
                        xch[:, b * BLK : (b + 1) * BLK],
                        start=True,
                        stop=True,
                    )
                    osl = och[:, b * BLK : (b + 1) * BLK]
                    if b % 2 == 0:
                        nc.scalar.activation(
                            osl, po[:], mybir.ActivationFunctionType.Copy
                        )
                    else:
                        nc.vector.tensor_scalar_add(osl, po[:], 0.0)
                    if b == half - 1:
                        nc.sync.dma_start(
                            out_ap[:, c0 * BLK : (c0 + half) * BLK],
                            och[:, : half * BLK],
                        )
                nc.sync.dma_start(
                    out_ap[:, (c0 + half) * BLK : (c0 + bc) * BLK],
                    och[:, half * BLK : bc * BLK],
                )
    nc.compile()
    _PROGRAM_CACHE[n_blocks] = nc
    return nc


def kernel(r_trial, r_knots, coefficients, h, rmax):
    r = np.ascontiguousarray(np.asarray(r_trial, np.float32))
    rk = np.asarray(r_knots, np.float32)
    coef = np.asarray(coefficients, np.float32)          # [127, 4, 128]
    h32 = np.float32(h)
    rmax32 = np.float32(rmax)
    n = r.shape[0]

    total_pad = N_CORES * NC_PAD
    rp = np.zeros(total_pad, np.float32)
    rp[:n] = r
    valid = np.zeros(total_pad, bool)
    valid[:n] = r < rmax32

    # segment + normalized local offset, reference float32 semantics
    t = (rp - rk[0]) / h32
    idx = np.clip(np.floor(t).astype(np.int32), 0, N_SEG - 1)
    dx = rp - rk[idx]
    u = dx / h32                                         # in [0, ~1]

    # h-scaled coefficients so every matmul operand is O(1):
    # out = a + (b*h)*u + (c*h^2)*u^2 + (d*h^3)*u^3
    hk = np.array([1.0, float(h32), float(h32) ** 2, float(h32) ** 3])
    coef_s = (coef.astype(np.float64) * hk[None, :, None]).astype(np.float32)

    nc = _build_program(BLOCKS)

    bcol = np.arange(NC_PAD, dtype=np.int64) // BLK      # block id per column
    in_maps = []
    orders = []
    uncovered_all = []
    for i in range(N_CORES):
        sl = slice(i * NC_PAD, (i + 1) * NC_PAD)
        idx_i, u_i, val_i = idx[sl], u[sl], valid[sl]
        key = np.where(val_i, idx_i, np.int32(1000))     # invalid/pad sort last
        order = np.argsort(key, kind="stable")
        sidx = idx_i[order]
        su = u_i[order]
        sval = val_i[order]

        segA = sidx[0::BLK]                              # [BLOCKS]
        segB = sidx[BLK - 1 :: BLK]
        mB = sval & (sidx == segB[bcol])
        mA = sval & ~mB & (sidx == segA[bcol]) & (segB[bcol] == segA[bcol] + 1)
        uncovered = sval & ~mA & ~mB          # >2 segs or non-adjacent
        uncovered_all.append(np.flatnonzero(uncovered))

        ok = mA | mB
        # z = u - (segB - seg) : 0-based from segment B's left knot
        z = np.where(ok, su + (sidx - segB[bcol]).astype(np.float32), 0.0)
        z = z.astype(np.float32)
        zm = np.minimum(z, np.float32(0))
        x5 = np.empty((KDIM, NC_PAD), np.float32)
        x5[0] = ok
        x5[1] = z
        x5[2] = z * z
        x5[3] = x5[2] * z
        x5[4] = zm * zm * zm

        cB = coef_s[segB]                                # [BLOCKS, 4, 128]
        w5 = np.empty((KDIM, BLOCKS, 128), np.float32)
        w5[0:4] = cB.transpose(1, 0, 2)
        w5[4] = coef_s[segA, 3] - cB[:, 3]               # (dA-dB)*h^3

        in_maps.append(
            {
                "x": x5.astype(np.float16),
                "w": w5.reshape(KDIM, BLOCKS * 128).astype(np.float16),
            }
        )
        orders.append(order)

    from concourse.bass_utils import run_bass_kernel_spmd

    res = run_bass_kernel_spmd(nc, in_maps, core_ids=list(range(N_CORES)))

    full = np.empty((total_pad, 128), np.float32)
    for i in range(N_CORES):
        shard = np.empty((NC_PAD, 128), np.float32)
        shard[orders[i]] = res.results[i]["out"].T.astype(np.float32)
        full[i * NC_PAD : (i + 1) * NC_PAD] = shard

    # exact host fixup for trials the device had to zero-mask (rare/never)
    for i in range(N_CORES):
        unc = uncovered_all[i]
        if unc.size:
            g = i * NC_PAD + orders[i][unc]  # original positions
            ri = rp[g]
            ii = idx[g]
            di = dx[g][:, None]
            cf = coef[ii]
            o = cf[:, 0] + di * (cf[:, 1] + di * (cf[:, 2] + di * cf[:, 3]))
            o[ri >= rmax32] = 0.0
            full[g] = o

    return full[:n]
